# revision 1
# baseline (speedup 1.0000x reference)
"""Chamfer L2 distance kernel for 8 Trainium2 NeuronCores.

Strategy (data-parallel over batch, 2 batches/core):
  For each batch and each direction (pred->target, target->pred) the device
  computes rowmin[n] = min_m H[n, m] where H = -2<x_n, y_m> + |y_m|^2 via
  K=21 bf16 matmuls (an exact hi/mid/lo bf16 decomposition of the fp32
  inputs, error ~1e-7 absolute) and a fused custom DVE min/min-reduce that
  consumes two fresh 1024-wide PSUM/SBUF tiles per pass.  The host adds the
  partition-side norms |x_n|^2 and finishes the means in fp64.

Self-contained: hardcodes B=16, N=M=4096, C=3, 8 cores.
"""

import numpy as np
import ml_dtypes

BF = ml_dtypes.bfloat16
B, N, M, C = 16, 4096, 4096, 3
NCORES = 8
BPC = B // NCORES          # batches per core
NU = BPC * 2               # (batch, orientation) units per core
K = 21                     # contraction rows (18 product terms + 3 norm rows)
NT = N // 128              # n-tiles per unit
SLOTS = NU * NT * 2        # accum slots (2 m-halves per n-tile)

_CACHE = {}


# ---------------------------------------------------------------- host prep --

def _split3(v):
    """Exact-ish 3-way bf16 decomposition: h + m + l = v + O(2^-27 |v|)."""
    h = v.astype(BF)
    r = v - h.astype(np.float64)
    m = r.astype(BF)
    r2 = r - m.astype(np.float64)
    l = r2.astype(BF)
    return h, m, l


def _build_tabs(X, Y):
    """X: (N,3) partition side, Y: (M,3) free side.
    Returns lhsT (21, N) bf16 and rhs (21, M) bf16 such that
    (lhsT.T @ rhs)[n, m] ~= -2<X_n, Y_m> + |Y_m|^2 to ~1e-7 absolute."""
    lt = np.empty((K, X.shape[0]), BF)
    rt = np.empty((K, Y.shape[0]), BF)
    Xd = X.astype(np.float64)
    Yd = -2.0 * Y.astype(np.float64)
    row = 0
    for c in range(C):
        Xh, Xm, Xl = _split3(Xd[:, c])
        Yh, Ym, Yl = _split3(Yd[:, c])
        for a, b in ((Xh, Yh), (Xh, Ym), (Xm, Yh), (Xm, Ym), (Xh, Yl), (Xl, Yh)):
            lt[row] = a
            rt[row] = b
            row += 1
    q = np.sum(Y.astype(np.float64) ** 2, axis=1)
    qh, qm, ql = _split3(q)
    ones = np.ones(X.shape[0], BF)
    for qq in (qh, qm, ql):
        lt[row] = ones
        rt[row] = qq
        row += 1
    assert row == K
    return lt, rt


# ------------------------------------------------------------- device build --

def _get_min_min_op():
    if "op" in _CACHE:
        return _CACHE["op"]
    import concourse.dve_ops as dve_ops_mod
    from concourse.dve_ops import DveOp
    from concourse.dve_spec import Spec, Src0, Src1, C0, minn, lower, _has_src1
    from concourse.dve_uop import DveOpSpec

    name = "CHAMFER_MIN_MIN_ANT"
    for op in dve_ops_mod.OPS:
        if op.name == name:
            _CACHE["op"] = op
            return op
    spec = Spec(
        body=minn(Src0, Src1),
        accum=minn,
        accum_init=C0,
        reference=lambda in0, in1, s0, s1, imm2: (
            (b := np.minimum(in0.astype(np.float32), in1.astype(np.float32))),
            np.minimum(
                b.reshape(b.shape[0], -1).min(axis=-1, keepdims=True),
                np.asarray(s0, np.float32).reshape(-1, 1),
            ),
        ),
    )
    if name not in dve_ops_mod._SUB_OPCODE_FOR_NAME:
        row = max(dve_ops_mod._SUB_OPCODE_FOR_NAME.values()) + 1
        assert row < 0x20
        dve_ops_mod._SUB_OPCODE_FOR_NAME[name] = row
    shas = {}
    for ver in ("v3", "v4"):
        try:
            s = DveOpSpec(
                name=name,
                opcode=dve_ops_mod.get_dve_sub_opcode(name),
                uops=lower(spec, ver=ver),
                rd1_en=_has_src1(spec),
            )
            shas[ver] = s.sha(ver)
        except Exception:
            pass
    op = DveOp(name, spec, False, shas)
    dve_ops_mod.OPS.append(op)
    dve_ops_mod.CUSTOM_DVE_SPECS[name] = spec
    _CACHE["op"] = op
    return op


def _build_nc(reps=1):
    key = ("nc", reps)
    if key in _CACHE:
        return _CACHE[key]
    import concourse.bacc as bacc
    import concourse.mybir as mybir
    from concourse.tile import TileContext

    MIN_MIN = _get_min_min_op()
    f32 = mybir.dt.float32
    bf16 = mybir.dt.bfloat16

    nc = bacc.Bacc(None)
    ltab = nc.dram_tensor("ltab", [NU, K, N], bf16, kind="ExternalInput")
    rtab = nc.dram_tensor("rtab", [NU, K, M], bf16, kind="ExternalInput")
    outt = nc.dram_tensor("out", [128, SLOTS], f32, kind="ExternalOutput")

    with TileContext(nc) as tc:
        with (
            tc.tile_pool(name="stage", bufs=2) as stage,
            tc.tile_pool(name="psum", bufs=2, space="PSUM") as psum,
            tc.tile_pool(name="cpp", bufs=3) as cpp,
            tc.tile_pool(name="res", bufs=1) as res,
        ):
            raw = res.tile([128, SLOTS], f32, tag="raw")
            # rotate the discarded broadcast-out target so consecutive DVE ops
            # have no WAW on the same tile
            dummies = [res.tile([128, 1], f32, tag=f"dummy{d}", name=f"dummy{d}")
                       for d in range(4)]
            for _rep in range(reps):
              for u in range(NU):
                  lt = stage.tile([K, N], bf16, tag="lt")
                  rt = stage.tile([K, M], bf16, tag="rt")
                  nc.sync.dma_start(out=lt[:, :], in_=ltab[u])
                  nc.sync.dma_start(out=rt[:, :], in_=rtab[u])
                  for i in range(NT):
                      ltT = lt[:, i * 128:(i + 1) * 128]
                      for h in range(2):
                          pa = psum.tile([128, 1024], f32, tag="pa")
                          pb = psum.tile([128, 1024], f32, tag="pb")
                          base = h * 2048
                          nc.tensor.matmul(pb[:, 0:512], ltT, rt[:, base + 1024:base + 1536],
                                           start=True, stop=True)
                          nc.tensor.matmul(pb[:, 512:1024], ltT, rt[:, base + 1536:base + 2048],
                                           start=True, stop=True)
                          nc.tensor.matmul(pa[:, 0:512], ltT, rt[:, base:base + 512],
                                           start=True, stop=True)
                          nc.tensor.matmul(pa[:, 512:1024], ltT, rt[:, base + 512:base + 1024],
                                           start=True, stop=True)
                          cp = cpp.tile([128, 1024], f32, tag="cp")
                          nc.scalar.copy(out=cp[:, :], in_=pb[:, :])
                          slot = (u * NT + i) * 2 + h
                          nc.vector._custom_dve(
                              MIN_MIN,
                              out=dummies[slot % 4].broadcast_to(pa[:, :].shape),
                              in0=pa[:, :],
                              in1=cp[:, :],
                              s0=1.0e30,
                              accum_out=raw[:, slot:slot + 1],
                          )
            nc.sync.dma_start(out=outt[:, :], in_=raw[:, :])
    nc.compile()
    _CACHE[key] = nc
    return nc


# -------------------------------------------------------------------- entry --

def _prepare_inputs(pred, target):
    ltabs = np.empty((NCORES, NU, K, N), BF)
    rtabs = np.empty((NCORES, NU, K, M), BF)
    for core in range(NCORES):
        for lb in range(BPC):
            b = core * BPC + lb
            for o in range(2):
                X = pred[b] if o == 0 else target[b]
                Y = target[b] if o == 0 else pred[b]
                lt, rt = _build_tabs(X, Y)
                u = lb * 2 + o
                ltabs[core, u] = lt
                rtabs[core, u] = rt
    return ltabs, rtabs


def _postprocess(results, pred, target):
    losses = []
    for core in range(NCORES):
        out = np.asarray(results[core]["out"])  # (128, SLOTS)
        for lb in range(BPC):
            b = core * BPC + lb
            total = 0.0
            for o in range(2):
                u = lb * 2 + o
                sl = out[:, u * (NT * 2):(u + 1) * (NT * 2)]
                rowmin = sl.reshape(128, NT, 2).min(axis=2)      # (p, i)
                rowmin = rowmin.T.reshape(-1)                     # n = i*128 + p
                X = pred[b] if o == 0 else target[b]
                s2 = np.sum(X.astype(np.float64) ** 2, axis=1)
                total += (s2 + rowmin).mean()
            losses.append(total)
    return np.float32(np.mean(losses))


def _run(pred, target, trace=False):
    from concourse.bass_utils import run_bass_kernel_spmd

    pred = np.asarray(pred, dtype=np.float32)
    target = np.asarray(target, dtype=np.float32)
    assert pred.shape == (B, N, C) and target.shape == (B, M, C)
    ltabs, rtabs = _prepare_inputs(pred, target)
    nc = _build_nc()
    in_maps = [{"ltab": ltabs[c], "rtab": rtabs[c]} for c in range(NCORES)]
    res = run_bass_kernel_spmd(nc, in_maps, core_ids=list(range(NCORES)), trace=trace)
    return _postprocess(res.results, pred, target), res


def kernel(pred, target):
    loss, _ = _run(pred, target, trace=False)
    return loss



# revision 7
# speedup vs baseline: 8.9075x; 8.9075x over previous
"""Chamfer L2 distance kernel for 8 Trainium2 NeuronCores — banded KNN version.

Strategy (data-parallel over batch, 2 batches/core, 2 directions/batch):
  Both point clouds are sorted by their z coordinate on the host. Row-tile i
  (128 consecutive sorted X points) only computes distances against a W=512
  window of sorted Y centered at the matching rank — a banded slice of the
  full 4096x4096 distance matrix (8x less work). A per-row certificate
  (banded_rowmin <= gap^2, where gap is the z distance to the nearest
  excluded point) proves the banded min is the true min; the handful of
  uncertified rows (~0.3%) are recomputed exactly on the host.

  The banded H[n, m] = -2<x_n, y_m> + |y_m|^2 is computed via K=21 bf16
  matmuls (exact hi/mid/lo bf16 decomposition, ~1e-7 error) with weights
  rotated across 4 PE row-groups so LDWEIGHTS overlaps matmuls. Row-mins
  come from the fused custom DVE min/min-reduce on PSUM tile halves, with
  ScalarE copying the partner half to SBUF (DVE allows only one PSUM
  operand). The host adds |x_n|^2, certifies, patches, and averages.

Self-contained: hardcodes B=16, N=M=4096, C=3, 8 cores.
"""

import numpy as np
import ml_dtypes

BF = ml_dtypes.bfloat16
B, N, M, C = 16, 4096, 4096, 3
NCORES = 8
BPC = B // NCORES          # batches per core
NU = BPC * 2               # (batch, direction) units per core
K = 21                     # contraction rows (18 product terms + 3 norm rows)
NT = N // 128              # n-tiles per unit
W = 512                    # band width (columns per n-tile)
TPS = 4                    # tiles per PSUM strip (strip = [128, TPS*W] = 4 banks)
NGROUPS = 3                # PE row-group rotation (base partition 96 unsupported)
CERT_SLACK = 2e-5          # device numerics margin for the certificate
USE_SEGMIN = True          # one segmented DVE op per strip vs per-tile pair ops

_CACHE = {}


def _window_lo(i):
    """Static window start for tile i (sorted-rank space)."""
    c = 128 * i + 64
    return min(max(0, c - W // 2), M - W)


# ---------------------------------------------------------------- host prep --

def _split3(v):
    """Exact-ish 3-way bf16 decomposition: h + m + l = v + O(2^-27 |v|)."""
    h = v.astype(BF)
    r = v - h.astype(np.float64)
    m = r.astype(BF)
    r2 = r - m.astype(np.float64)
    l = r2.astype(BF)
    return h, m, l


def _build_tabs(X, Y):
    """X: (N,3) partition side, Y: (M,3) free side.
    Returns lhsT (21, N) bf16 and rhs (21, M) bf16 such that
    (lhsT.T @ rhs)[n, m] ~= -2<X_n, Y_m> + |Y_m|^2 to ~1e-7 absolute."""
    lt = np.empty((K, X.shape[0]), BF)
    rt = np.empty((K, Y.shape[0]), BF)
    Xd = X.astype(np.float64)
    Yd = -2.0 * Y.astype(np.float64)
    row = 0
    for c in range(C):
        Xh, Xm, Xl = _split3(Xd[:, c])
        Yh, Ym, Yl = _split3(Yd[:, c])
        for a, b in ((Xh, Yh), (Xh, Ym), (Xm, Yh), (Xm, Ym), (Xh, Yl), (Xl, Yh)):
            lt[row] = a
            rt[row] = b
            row += 1
    q = np.sum(Y.astype(np.float64) ** 2, axis=1)
    qh, qm, ql = _split3(q)
    ones = np.ones(X.shape[0], BF)
    for qq in (qh, qm, ql):
        lt[row] = ones
        rt[row] = qq
        row += 1
    assert row == K
    return lt, rt


def _unit_xy(pred, target, b, o):
    X = pred[b] if o == 0 else target[b]
    Y = target[b] if o == 0 else pred[b]
    return X, Y


def _sort_perm(P):
    return np.argsort(P[:, 2], kind="stable")


# ------------------------------------------------------------- device build --

def _get_min_min_op():
    if "op" in _CACHE:
        return _CACHE["op"]
    import concourse.dve_ops as dve_ops_mod
    from concourse.dve_ops import DveOp
    from concourse.dve_spec import Spec, Src0, Src1, C0, minn, lower, _has_src1
    from concourse.dve_uop import DveOpSpec

    name = "CHAMFER_MIN_MIN_ANT"
    for op in dve_ops_mod.OPS:
        if op.name == name:
            _CACHE["op"] = op
            return op
    spec = Spec(
        body=minn(Src0, Src1),
        accum=minn,
        accum_init=C0,
        reference=lambda in0, in1, s0, s1, imm2: (
            (b := np.minimum(in0.astype(np.float32), in1.astype(np.float32))),
            np.minimum(
                b.reshape(b.shape[0], -1).min(axis=-1, keepdims=True),
                np.asarray(s0, np.float32).reshape(-1, 1),
            ),
        ),
    )
    if name not in dve_ops_mod._SUB_OPCODE_FOR_NAME:
        row = max(dve_ops_mod._SUB_OPCODE_FOR_NAME.values()) + 1
        assert row < 0x20
        dve_ops_mod._SUB_OPCODE_FOR_NAME[name] = row
    shas = {}
    for ver in ("v3", "v4"):
        try:
            s = DveOpSpec(
                name=name,
                opcode=dve_ops_mod.get_dve_sub_opcode(name),
                uops=lower(spec, ver=ver),
                rd1_en=_has_src1(spec),
            )
            shas[ver] = s.sha(ver)
        except Exception:
            pass
    op = DveOp(name, spec, False, shas)
    dve_ops_mod.OPS.append(op)
    dve_ops_mod.CUSTOM_DVE_SPECS[name] = spec
    _CACHE["op"] = op
    return op


def _get_segmin_op():
    """Segmented row-min op: in0/in1 are [128, S, H] (S segments of H
    columns); body = running min (reset at each segment boundary) of
    min(src0, src1). The destination AP repeats each segment slot H times
    (inner stride 0), so the last write per segment — the segment's min —
    is what lands: out[:, s] = min over the segment. No accumulator read."""
    if "segop" in _CACHE:
        return _CACHE["segop"]
    import dataclasses
    import concourse.dve_ops as dve_ops_mod
    from concourse.dve_ops import DveOp
    import concourse.dve_spec as dve_spec
    from concourse.dve_spec import (
        Spec, Src0, Src1, C0, minn, lower, _has_src1, Scan, AluOp,
    )
    from concourse.dve_uop import DveOpSpec

    name = "CHAMFER_SEGMIN_ANT"
    for op in dve_ops_mod.OPS:
        if op.name == name:
            _CACHE["segop"] = op
            return op

    @dataclasses.dataclass(frozen=True)
    class ResetScan(Scan):
        """Scan that re-seeds from `init` at each SUB_DIM_DONE."""
        _reset_at_subdim = True  # class marker, not a dataclass field

    if not getattr(dve_spec, "_chamfer_reset_patch", False):
        _orig_scan_overrides = dve_spec._scan_overrides

        def _patched_scan_overrides(scans, node_stage):
            seed, step = _orig_scan_overrides(scans, node_stage)
            for sc in scans:
                if getattr(sc, "_reset_at_subdim", False):
                    d = node_stage[sc]
                    step[d] = dve_spec._Stage(
                        sc.op, dve_spec._scan_init(sc), sc.expr)
            return seed, step

        dve_spec._scan_overrides = _patched_scan_overrides
        dve_spec._chamfer_reset_patch = True

    def ref(in0, in1, s0, s1, imm2):
        a = np.minimum(np.asarray(in0, np.float32), np.asarray(in1, np.float32))
        if a.ndim == 2:
            a = a[:, None, :]
        seg = a.min(axis=-1, keepdims=True)
        seg = np.minimum(seg, np.asarray(s0, np.float32).reshape(-1, 1, 1))
        # broadcast so the final memory state matches regardless of the
        # simulator's write order through the stride-0 destination
        return np.broadcast_to(seg, a.shape).copy().reshape(np.shape(in0))

    spec = Spec(
        body=ResetScan(AluOp.MIN, minn(Src0, Src1), init=C0),
        reference=ref,
    )
    if name not in dve_ops_mod._SUB_OPCODE_FOR_NAME:
        row = max(dve_ops_mod._SUB_OPCODE_FOR_NAME.values()) + 1
        assert row < 0x20
        dve_ops_mod._SUB_OPCODE_FOR_NAME[name] = row
    shas = {}
    for ver in ("v3", "v4"):
        try:
            s = DveOpSpec(
                name=name,
                opcode=dve_ops_mod.get_dve_sub_opcode(name),
                uops=lower(spec, ver=ver),
                rd1_en=_has_src1(spec),
            )
            shas[ver] = s.sha(ver)
        except Exception:
            pass
    op = DveOp(name, spec, True, shas)   # subdim=True
    dve_ops_mod.OPS.append(op)
    dve_ops_mod.CUSTOM_DVE_SPECS[name] = spec
    _CACHE["segop"] = op
    return op


def _build_nc(reps=1):
    key = ("nc", reps)
    if key in _CACHE:
        return _CACHE[key]
    import concourse.bacc as bacc
    import concourse.mybir as mybir
    from concourse.tile import TileContext

    MIN_MIN = _get_min_min_op()
    SEGMIN = _get_segmin_op() if USE_SEGMIN else None
    f32 = mybir.dt.float32
    bf16 = mybir.dt.bfloat16
    H = W // 2                 # half-window for the DVE pair trick
    NS = NT // TPS             # strips per unit

    nc = bacc.Bacc(None)
    ltab = nc.dram_tensor("ltab", [NU, K, N], bf16, kind="ExternalInput")
    rtab = nc.dram_tensor("rtab", [NU, K, M], bf16, kind="ExternalInput")
    outt = nc.dram_tensor("out", [128, NU * NT], f32, kind="ExternalOutput")

    with TileContext(nc) as tc:
        with (
            tc.tile_pool(name="stage", bufs=2) as stage,
            tc.tile_pool(name="psum", bufs=2, space="PSUM") as psum,
            tc.tile_pool(name="cpp", bufs=3) as cpp,
            tc.tile_pool(name="res", bufs=1) as res,
        ):
            raw = res.tile([128, NU * NT], f32, tag="raw")
            dummies = [res.tile([128, 1], f32, tag=f"dummy{d}", name=f"dummy{d}")
                       for d in range(4)]
            for _rep in range(reps):
              for u in range(NU):
                lt = stage.tile([128, N], bf16, tag="lt", name="lt")
                rt = stage.tile([128, M], bf16, tag="rt", name="rt")
                for g in range(NGROUPS):
                    nc.sync.dma_start(out=lt[32 * g:32 * g + K, :], in_=ltab[u])
                    nc.sync.dma_start(out=rt[32 * g:32 * g + K, :], in_=rtab[u])
                for s in range(NS):
                    strip = psum.tile([128, TPS * W], f32, tag="strip", name="strip")
                    for j in range(TPS):
                        i = s * TPS + j
                        g = 32 * (i % NGROUPS)
                        lo = _window_lo(i)
                        nc.tensor.matmul(
                            strip[:, W * j:W * (j + 1)],
                            lt[g:g + K, 128 * i:128 * (i + 1)],
                            rt[g:g + K, lo:lo + W],
                            start=True, stop=True)
                    cp = cpp.tile([128, TPS * H], f32, tag="cp", name="cp")
                    if USE_SEGMIN:
                        strip3 = strip[:, :].rearrange("p (s w) -> p s w", w=W)
                        cp3 = cp[:, :].rearrange("p (s h) -> p s h", h=H)
                        nc.scalar.copy(out=cp3, in_=strip3[:, :, H:W])
                        slot0 = u * NT + s * TPS
                        nc.vector._custom_dve(
                            SEGMIN,
                            out=raw[:, slot0:slot0 + TPS]
                                .unsqueeze(-1).broadcast_to((128, TPS, H)),
                            in0=strip3[:, :, 0:H],
                            in1=cp3,
                            s0=1.0e30,
                        )
                    else:
                        for j in range(TPS):
                            nc.scalar.copy(out=cp[:, H * j:H * (j + 1)],
                                           in_=strip[:, W * j + H:W * (j + 1)])
                        for j in range(TPS):
                            i = s * TPS + j
                            slot = u * NT + i
                            nc.vector._custom_dve(
                                MIN_MIN,
                                out=dummies[slot % 4].broadcast_to(cp[:, :H].shape),
                                in0=strip[:, W * j:W * j + H],
                                in1=cp[:, H * j:H * (j + 1)],
                                s0=1.0e30,
                                accum_out=raw[:, slot:slot + 1],
                            )
            nc.sync.dma_start(out=outt[:, :], in_=raw[:, :])
    nc.compile()
    _CACHE[key] = nc
    return nc


# -------------------------------------------------------------------- entry --

def _prepare_inputs(pred, target):
    ltabs = np.empty((NCORES, NU, K, N), BF)
    rtabs = np.empty((NCORES, NU, K, M), BF)
    for core in range(NCORES):
        for lb in range(BPC):
            b = core * BPC + lb
            for o in range(2):
                X, Y = _unit_xy(pred, target, b, o)
                Xs = X[_sort_perm(X)]
                Ys = Y[_sort_perm(Y)]
                lt, rt = _build_tabs(Xs, Ys)
                u = lb * 2 + o
                ltabs[core, u] = lt
                rtabs[core, u] = rt
    return ltabs, rtabs


def _postprocess(results, pred, target):
    losses = []
    n_fallback = 0
    for core in range(NCORES):
        out = np.asarray(results[core]["out"])  # (128, NU*NT)
        for lb in range(BPC):
            b = core * BPC + lb
            total = 0.0
            for o in range(2):
                u = lb * 2 + o
                X, Y = _unit_xy(pred, target, b, o)
                px = _sort_perm(X)
                py = _sort_perm(Y)
                Xs = X[px].astype(np.float64)
                Ys = Y[py].astype(np.float64)
                kx = Xs[:, 2]
                ky = Ys[:, 2]

                sl = out[:, u * NT:(u + 1) * NT]          # (128, NT), [p, i]
                hmin = sl.T.reshape(-1).astype(np.float64)  # n = 128*i + p
                # |x~|^2 from the exact bf16 splits used on device
                xt = np.zeros_like(Xs)
                for c in range(C):
                    h, m, l = _split3(Xs[:, c])
                    xt[:, c] = (h.astype(np.float64) + m.astype(np.float64)
                                + l.astype(np.float64))
                rowmin = hmin + np.sum(xt * xt, axis=1)

                # certificate: distance to nearest excluded z
                g = np.full(N, np.inf)
                for i in range(NT):
                    rows = slice(128 * i, 128 * i + 128)
                    lo = _window_lo(i)
                    glo = kx[rows] - ky[lo] if lo > 0 else np.inf
                    ghi = ky[lo + W - 1] - kx[rows] if lo + W < M else np.inf
                    g[rows] = np.minimum(glo, ghi)
                bad = rowmin > g * g - CERT_SLACK
                if bad.any():
                    n_fallback += int(bad.sum())
                    d = ((Xs[bad, None, :] - Ys[None, :, :]) ** 2).sum(-1)
                    rowmin[bad] = d.min(axis=1)
                total += rowmin.mean()
            losses.append(total)
    _CACHE["n_fallback"] = n_fallback
    return np.float32(np.mean(losses))


def _run(pred, target, trace=False):
    from concourse.bass_utils import run_bass_kernel_spmd

    pred = np.asarray(pred, dtype=np.float32)
    target = np.asarray(target, dtype=np.float32)
    assert pred.shape == (B, N, C) and target.shape == (B, M, C)
    ltabs, rtabs = _prepare_inputs(pred, target)
    nc = _build_nc()
    in_maps = [{"ltab": ltabs[c], "rtab": rtabs[c]} for c in range(NCORES)]
    res = run_bass_kernel_spmd(nc, in_maps, core_ids=list(range(NCORES)), trace=trace)
    return _postprocess(res.results, pred, target), res


def kernel(pred, target):
    loss, _ = _run(pred, target, trace=False)
    return loss
